# revision 27
# baseline (speedup 1.0000x reference)
"""Trainium2 Bass kernel for DenseConv2d.

Conv2d: input (32,128,56,56) f32, weight (256,128,3,3) f32, bias (256,) f32,
stride 1, pad 1, dilation 1 -> output (32,256,56,56) f32.

Strategy: data-parallel over batch across 8 NeuronCores (4 images per core).
Per core the conv is computed as 9 accumulated matmuls (one per kernel tap)
into PSUM: out[co, pix] += W[kh,kw][ci,co].T @ x_pad[ci, shifted pix window].
Operands stream through the PE array as float32r (~1.1 cycles/row sustained).
Input is chunked (2 row-blocks + halo per DMA) on the scalar-engine HWDGE
queue so the first matmul starts as early as possible; output DMAs ride the
sync queue. A few warmup matmuls on scratch data run during the input DMA
wait to lift the PE HAM clock-gate to 2.4 GHz before real work arrives.
Layout prep (padding, channel-major transpose) is host-side numpy.
"""

import sys

if "/opt/trn_rl_repo" not in sys.path:
    sys.path.insert(0, "/opt/trn_rl_repo")

import numpy as np

N_CORES = 8
N, CI, H, W = 32, 128, 56, 56
CO, KH, KW = 256, 3, 3
NP_CORE = N // N_CORES          # images per core
HP, WP = H + 2, W + 2           # padded spatial dims
COT = CO // 128                 # out-channel tiles of 128
RB = 8                          # output rows per matmul block
NBLK = H // RB                  # row blocks per image
CHROWS = 2 * RB + 2             # input rows per chunk (2 blocks + halo)
NCH = 4                         # chunks per image (last one is short)
N_WARMUP = 7                    # PE warmup matmuls

_CACHE = {}


def _build_program():
    import concourse.mybir as mybir
    from concourse import bacc
    from concourse.tile import TileContext

    nc = bacc.Bacc(None, target_bir_lowering=False)

    x_d = nc.dram_tensor("x", [CI, NP_CORE, HP, WP], mybir.dt.float32r,
                         kind="ExternalInput")
    w_d = nc.dram_tensor("w", [CI, COT, KH * KW, 128], mybir.dt.float32r,
                         kind="ExternalInput")
    b_d = nc.dram_tensor("b2", [128, COT], mybir.dt.float32,
                         kind="ExternalInput")
    y_d = nc.dram_tensor("y", [COT, 128, NP_CORE, H, W], mybir.dt.float32,
                         kind="ExternalOutput")

    f32 = mybir.dt.float32
    f32r = mybir.dt.float32r

    with TileContext(nc) as tc:
        with (
            tc.tile_pool(name="xin", bufs=1) as xpool,
            tc.tile_pool(name="wpool", bufs=1) as wpool,
            tc.tile_pool(name="bpool", bufs=1) as bpool,
            tc.tile_pool(name="psum", bufs=8, space="PSUM") as ppool,
            tc.tile_pool(name="out", bufs=6) as opool,
        ):
            # PE warmup on scratch data, concurrent with the input DMAs,
            # so the HAM clock-gate is at 2.4 GHz when real matmuls start.
            scratch = xpool.tile([CI, RB * W], mybir.dt.bfloat16,
                                 tag="scratch")
            nc.vector.memset(scratch, 0.0)
            wups = ppool.tile([128, RB * W], f32, tag="ps")
            for _ in range(N_WARMUP):
                nc.tensor.matmul(wups, scratch[:, 0:128], scratch,
                                 start=True, stop=True)
            # Tiny-warmup tail: keeps the PE continuously busy (HAM stays
            # ramping) until the first real operands arrive, while adding
            # at most ~0.1 us of queue delay ahead of the real matmuls.
            for _ in range(10):
                nc.tensor.matmul(wups[:, 0:64], scratch[:, 0:128],
                                 scratch[:, 0:64], start=True, stop=True)

            # Weights split by out-channel tile so the first matmul group
            # only waits for w[cot=0] + the first input chunk (~0.7 MB).
            wt = []
            for cot in range(COT):
                wtile = wpool.tile([CI, KH * KW, 128], f32r, tag=f"w{cot}")
                wt.append(wtile)
            bt = bpool.tile([128, COT], f32)

            def wslice(pos, cot):
                return wt[cot][:, pos, :]

            # Input chunks per image: (padded_row0, n_blocks). The first is
            # a single block so the very first matmul group's data arrives
            # fast; block b lives in chunk CHMAP[b] at local row CHLOC[b].
            CHUNKS = [(0, 1), (RB, 2), (3 * RB, 2), (5 * RB, 2)]
            CHMAP, CHLOC = {}, {}
            b = 0
            for ci_, (r0_, nb_) in enumerate(CHUNKS):
                for j in range(nb_):
                    CHMAP[b], CHLOC[b] = ci_, j * RB
                    b += 1
            xt = {}

            def x_chunk_dma(img, ch, eng):
                r0, nb = CHUNKS[ch]
                rows = min(nb * RB + 2, HP - r0)
                t = xpool.tile([CI, rows, WP], f32r, tag=f"x{img}_{ch}")
                eng.dma_start(out=t, in_=x_d[:, img, r0:r0 + rows, :])
                xt[img, ch] = t

            # Critical path: the first matmul group needs x0 chunk0 plus all
            # 9 taps of w[cot0]; spread those over both HWDGE queues.
            nc.scalar.dma_start(out=wt[0][:, 0:5, :], in_=w_d[:, 0, 0:5, :])
            x_chunk_dma(0, 0, nc.sync)
            nc.sync.dma_start(out=wt[0][:, 5:9, :], in_=w_d[:, 0, 5:9, :])
            x_chunk_dma(0, 1, nc.scalar)
            nc.sync.dma_start(out=wt[1], in_=w_d[:, 1, :, :])
            x_chunk_dma(0, 2, nc.scalar)
            x_chunk_dma(0, 3, nc.sync)
            nc.scalar.dma_start(out=bt, in_=b_d[:, :])
            for img in range(1, NP_CORE):
                for ch in range(len(CHUNKS)):
                    x_chunk_dma(img, ch, nc.scalar)

            for img in range(NP_CORE):
                for cot in range(COT):
                    for blk in range(NBLK):
                        ps = ppool.tile([128, RB, W], f32, tag="ps")
                        ch, r0 = CHMAP[blk], CHLOC[blk]
                        for pos in range(KH * KW):
                            kh, kw = divmod(pos, KW)
                            rhs = xt[img, ch][:, r0 + kh:r0 + kh + RB,
                                              kw:kw + W]
                            nc.tensor.matmul(
                                ps, wslice(pos, cot), rhs,
                                start=(pos == 0), stop=(pos == KH * KW - 1),
                            )
                        ot = opool.tile([128, RB, W], f32)
                        nc.vector.tensor_scalar_add(ot, ps, bt[:, cot:cot + 1])
                        nc.sync.dma_start(
                            out=y_d[cot, :, img, blk * RB:blk * RB + RB, :],
                            in_=ot)

    nc.compile()
    return nc


def prep_in_maps(input, weight, bias):
    """Host-side layout prep -> one in_map per core."""
    xp = np.pad(input, ((0, 0), (0, 0), (1, 1), (1, 1)))
    # weight [co, ci, kh, kw] -> [ci, cot, (kh kw), cop]
    wr = np.ascontiguousarray(
        weight.transpose(1, 2, 3, 0).reshape(CI, KH * KW, COT, 128)
        .transpose(0, 2, 1, 3))
    b2 = np.ascontiguousarray(bias.reshape(COT, 128).T)

    in_maps = []
    for c in range(N_CORES):
        xc = np.ascontiguousarray(
            xp[c * NP_CORE:(c + 1) * NP_CORE].transpose(1, 0, 2, 3))
        in_maps.append({"x": xc, "w": wr, "b2": b2})
    return in_maps


def kernel(input, weight, bias):
    input = np.asarray(input, dtype=np.float32)
    weight = np.asarray(weight, dtype=np.float32)
    bias = np.asarray(bias, dtype=np.float32)

    if "nc" not in _CACHE:
        _CACHE["nc"] = _build_program()
    nc = _CACHE["nc"]

    from concourse.bass_utils import run_bass_kernel_spmd

    in_maps = prep_in_maps(input, weight, bias)
    res = run_bass_kernel_spmd(nc, in_maps, core_ids=list(range(N_CORES)))

    out = np.empty((N, CO, H, W), dtype=np.float32)
    for c in range(N_CORES):
        y = res.results[c]["y"]  # [COT, 128, NP_CORE, H, W]
        out[c * NP_CORE:(c + 1) * NP_CORE] = (
            y.transpose(2, 0, 1, 3, 4).reshape(NP_CORE, CO, H, W))
    return out
